# revision 1
# baseline (speedup 1.0000x reference)
"""Causal multi-head attention (32 heads, seq=128, d_model=4096) on 8 TRN2 cores.

Sharding: tensor-parallel over heads. Core c owns heads 4c..4c+3, i.e. rows
512c:512(c+1) of Q/K/V and columns 512c:512(c+1) of O. Each core computes its
partial output O_c @ att_c as out^T (128, 4096) in bf16; the host sums the 8
partials in float64 and transposes back.

Design (143.6us fp32 baseline -> ~62-70us, HBM-wire-bound):
- Weights are downcast to bf16 on the host: the bytes in DRAM are what DMA
  must move, so shipping bf16 halves HBM traffic AND runs the big matmuls
  at 1 cycle/row instead of fp32's 4. Host-side rounding costs ~6e-3 rel
  err vs the 2e-2 gate (fp8 would be ~3.6% sigma -> fails; fp16's e^18
  overflows the softmax exp).
- All shards are stored partition-tiled in DRAM — w_pt[p, it*W + c] =
  w[it*128 + p, c] — so every DMA line is >= 4KB contiguous and one DMA
  loads up to 16 contraction tiles. 15 loads total: the framework rotates
  ~10 DMA data semaphores and a 3rd-generation reuse stalls the queue.
- Wire order = consume order: x/qt/kt interleaved, then vt (tapered
  16/12/4 so the last piece lands early), then ot per output chunk, with
  the final chunk's pieces tapering to a single head-slice so only one
  matmul + cast + store trail the last byte.
- The whole softmax is batched across the 4 heads: all masked scores land
  in ONE PSUM bank (causal mask generated on-device via gpsimd
  affine_select and folded in via an identity matmul), one exp, one
  segmented DVE row-sum, probabilities pre-normalized before transposing —
  no per-head cross-engine round-trips on the critical path.
- Output chunks pair into resident [128,1024] bf16 tiles (2KB lines) so
  stores never backpressure PSUM recycling; stores go on the SYNC queue,
  in-order behind all loads, keeping the load stream pure-read (no HBM
  write turnarounds mid-stream — measurably more fast-mode draws).
Remaining time is preamble (~8.2us framework barriers) + 17.8MB of loads
at ~360-400GB/s/core (run-to-run spread = HBM contention with the pair
core) + ~4.5us tail + ~2.2us teardown.
"""

import math
import sys

import ml_dtypes
import numpy as np

sys.path.insert(0, "/opt/trn_rl_repo")

import concourse.bacc as bacc
import concourse.bass as bass
import concourse.mybir as mybir
import concourse.tile as tile
from concourse.bass import ts
from concourse.bass_utils import run_bass_kernel_spmd
from concourse.masks import make_identity

P = 128
DM = 4096          # d_model
SEQ = 128
DK = 128           # head dim
NCORES = 8
HPC = 4            # heads per core
OW = HPC * DK      # 512: per-core projection width
KT = DM // P       # 32 contraction tiles
WB = 4             # weight DMA batches per tensor
KPB = KT // WB     # 8 contraction tiles per batch
NCHUNK = DM // OW  # 8 output chunks
F32 = mybir.dt.float32
BF16 = mybir.dt.bfloat16
SCALE = 1.0 / math.sqrt(DK)
BF = ml_dtypes.bfloat16
MASK_VAL = -1e30


def build_nc():
    nc = bacc.Bacc("TRN2", target_bir_lowering=False, debug=False)

    # partition-tiled bf16 weights: qtb[p, it*OW + c] = Q_shard^T[it*128+p, c]
    qtb = nc.dram_tensor("qtb", (P, KT * OW), BF16, kind="ExternalInput")
    ktb = nc.dram_tensor("ktb", (P, KT * OW), BF16, kind="ExternalInput")
    vtb = nc.dram_tensor("vtb", (P, KT * OW), BF16, kind="ExternalInput")
    # otb[p, c*2048 + h*512 + j] = O_shard^T[h*128+p, c*512+j]
    otb = nc.dram_tensor("otb", (P, NCHUNK * HPC * OW), BF16, kind="ExternalInput")
    xtb = nc.dram_tensor("xtb", (P, DM), BF16, kind="ExternalInput")
    out = nc.dram_tensor("out", (SEQ, DM), BF16, kind="ExternalOutput")

    with tile.TileContext(nc) as tc:
        with (
            tc.tile_pool(name="const", bufs=1) as cpool,
            tc.tile_pool(name="xtp", bufs=1) as xtp,
            tc.tile_pool(name="sb", bufs=1) as sb,
            tc.tile_pool(name="wts", bufs=1) as wts,
            tc.tile_pool(name="otp", bufs=1) as otp,
            tc.tile_pool(name="attn", bufs=2) as attnp,
            tc.tile_pool(name="outp", bufs=1) as outp,
        ):
            # ---- DMA program order == wire order on the sync queue.
            # Keep the load count small: the framework rotates ~8 DMA data
            # semaphores, and a 3rd-generation reuse of a semaphore stalls
            # the queue until the 2nd-generation DMA fully drains. 13 loads
            # stay within 2 generations.
            xt_sb = xtp.tile([P, DM], BF16)
            qt_b, kt_b, vt_b = [], [], []
            for b in range(2):
                nc.sync.dma_start(xt_sb[:, ts(b, DM // 2)], xtb[:, ts(b, DM // 2)])
                qt = wts.tile([P, KT * OW // 2], BF16, tag=f"qt{b}")
                nc.sync.dma_start(qt, qtb[:, ts(b, KT * OW // 2)])
                qt_b.append(qt)
                kt = wts.tile([P, KT * OW // 2], BF16, tag=f"kt{b}")
                nc.sync.dma_start(kt, ktb[:, ts(b, KT * OW // 2)])
                kt_b.append(kt)
            # vt tapers (16/8/4/4 ktiles) so the last piece lands ~0.5MB
            # before wire-end and the v->softmax@v chain finishes early
            vt_splits = [16, 12, 4]
            vt_off = 0
            for b, nkt in enumerate(vt_splits):
                vt = wts.tile([P, nkt * OW], BF16, tag=f"vt{b}")
                nc.sync.dma_start(
                    vt, vtb[:, vt_off * OW : (vt_off + nkt) * OW]
                )
                vt_b.append((vt_off, vt))
                vt_off += nkt
            # ot: 3x 2-chunk loads, a 1-chunk load, then the last chunk in
            # two halves so the tail's dependent work is minimal
            ot_c = []
            for g in range(3):
                ot2 = otp.tile([P, 2 * HPC * OW], BF16, tag=f"ot{g}")
                nc.sync.dma_start(ot2, otb[:, ts(g, 2 * HPC * OW)])
                ot_c.append(ot2[:, ts(0, HPC * OW)])
                ot_c.append(ot2[:, ts(1, HPC * OW)])
            ot6 = otp.tile([P, HPC * OW], BF16, tag="ot6")
            nc.sync.dma_start(ot6, otb[:, ts(6, HPC * OW)])
            ot_c.append(ot6)
            ot7 = otp.tile([P, HPC * OW], BF16, tag="ot7")
            base = 7 * HPC * OW
            nc.sync.dma_start(ot7[:, 0 : 2 * OW], otb[:, base : base + 2 * OW])
            nc.sync.dma_start(
                ot7[:, 2 * OW : 3 * OW], otb[:, base + 2 * OW : base + 3 * OW]
            )
            nc.sync.dma_start(
                ot7[:, 3 * OW : 4 * OW], otb[:, base + 3 * OW : base + 4 * OW]
            )
            ot_c.append(ot7)

            # on-device constants (gpsimd, overlaps the load stream)
            ident_bf = cpool.tile([P, P], BF16)
            make_identity(nc, ident_bf)
            # keep sk >= sq: cmask[sq, sk] = (sk - sq) >= 0 ? 0 : MASK_VAL
            cmask_bf = cpool.tile([P, P], BF16)
            nc.gpsimd.memset(cmask_bf, 0.0)
            nc.gpsimd.affine_select(
                out=cmask_bf,
                in_=cmask_bf,
                compare_op=mybir.AluOpType.is_ge,
                fill=MASK_VAL,
                base=0,
                pattern=[[1, P]],
                channel_multiplier=-1,
            )

            # PSUM: psC(2) + psV(1) + psA(2, freed) + psB(3) + psS(2) = 8 banks
            with (
                tc.tile_pool(name="psC", bufs=2, space="PSUM") as psC,
                tc.tile_pool(name="psV", bufs=1, space="PSUM") as psV,
            ):
                # ---- Phase 1a: q/k projections (q = x @ Qc^T etc.) ----
                with tc.tile_pool(name="psA", bufs=1, space="PSUM") as psA:
                    q_ps = psA.tile([P, OW], F32, tag="q")
                    k_ps = psA.tile([P, OW], F32, tag="k")
                    for it in range(KT):
                        b, j = it // (KT // 2), it % (KT // 2)
                        st, sp = it == 0, it == KT - 1
                        xts = xt_sb[:, ts(it, SEQ)]
                        nc.tensor.matmul(
                            q_ps, xts, qt_b[b][:, ts(j, OW)], start=st, stop=sp
                        )
                        nc.tensor.matmul(
                            k_ps, xts, kt_b[b][:, ts(j, OW)], start=st, stop=sp
                        )
                    # round q/k to bf16 on the way out of PSUM (scale into q)
                    q_sb = sb.tile([P, OW], BF16, tag="q_sb")
                    nc.vector.tensor_scalar_mul(q_sb, q_ps, SCALE)
                    k_sb = sb.tile([P, OW], BF16, tag="k_sb")
                    nc.vector.tensor_copy(k_sb, k_ps)

                v_ps = psV.tile([P, OW], F32, tag="v")
                with (
                    tc.tile_pool(name="psB", bufs=4, space="PSUM") as psB,
                    tc.tile_pool(name="psS", bufs=1, space="PSUM") as psS,
                ):
                    # ---- Phase 2a: batched attention prep (v-independent),
                    # before the v matmuls in program order so it fills PE
                    # idle slots while the vt batches stream in. The whole
                    # softmax runs as ONE exp + one segmented row-sum, so no
                    # per-head cross-engine round-trips.
                    qT, kT = [], []
                    for h in range(HPC):
                        tq_ps = psB.tile([P, P], BF16, tag="t")
                        nc.tensor.transpose(tq_ps, q_sb[:, ts(h, DK)], ident_bf)
                        qT_sb = attnp.tile([P, P], BF16, tag=f"qT{h}")
                        nc.vector.tensor_copy(qT_sb, tq_ps)
                        qT.append(qT_sb)
                        tk_ps = psB.tile([P, P], BF16, tag="t")
                        nc.tensor.transpose(tk_ps, k_sb[:, ts(h, DK)], ident_bf)
                        kT_sb = attnp.tile([P, P], BF16, tag=f"kT{h}")
                        nc.vector.tensor_copy(kT_sb, tk_ps)
                        kT.append(kT_sb)
                    # all 4 heads' masked scores into one PSUM bank
                    sc_all = psS.tile([P, OW], F32, tag="sc")
                    for h in range(HPC):
                        nc.tensor.matmul(
                            sc_all[:, ts(h, P)], ident_bf, cmask_bf,
                            start=True, stop=False,
                        )
                        nc.tensor.matmul(
                            sc_all[:, ts(h, P)], qT[h], kT[h],
                            start=False, stop=True,
                        )
                    # one exp over all heads (logits bounded ~|10|: softmax
                    # without max-subtraction is safe); rowsums on DVE
                    e_all = attnp.tile([P, OW], BF16, tag="e_all")
                    nc.scalar.activation(
                        e_all, sc_all, mybir.ActivationFunctionType.Exp
                    )
                    rs = attnp.tile([P, HPC], F32, tag="rs")
                    nc.vector.tensor_reduce(
                        rs,
                        e_all.rearrange("p (h k) -> p h k", h=HPC),
                        axis=mybir.AxisListType.X,
                        op=mybir.AluOpType.add,
                    )
                    recip = attnp.tile([P, HPC], F32, tag="recip")
                    nc.vector.reciprocal(recip, rs)
                    # pre-normalize probabilities per head, then transpose
                    en = attnp.tile([P, OW], BF16, tag="en")
                    pT = []
                    for h in range(HPC):
                        nc.vector.tensor_scalar_mul(
                            en[:, ts(h, P)], e_all[:, ts(h, P)], recip[:, ts(h, 1)]
                        )
                        pt_ps = psB.tile([P, P], BF16, tag="t")
                        nc.tensor.transpose(pt_ps, en[:, ts(h, P)], ident_bf)
                        pT_sb = attnp.tile([P, P], BF16, tag=f"pT{h}")
                        nc.vector.tensor_copy(pT_sb, pt_ps)
                        pT.append(pT_sb)

                    # ---- Phase 1b: v projection ----
                    for vt_off, vt in vt_b:
                        for j in range(vt.shape[1] // OW):
                            it = vt_off + j
                            st, sp = it == 0, it == KT - 1
                            nc.tensor.matmul(
                                v_ps,
                                xt_sb[:, ts(it, SEQ)],
                                vt[:, ts(j, OW)],
                                start=st,
                                stop=sp,
                            )
                    # per-head PSUM->bf16 casts so at_h fires without waiting
                    # the full v copy
                    v_sb = sb.tile([P, OW], BF16, tag="v_sb")
                    for h in range(HPC):
                        nc.vector.tensor_copy(
                            v_sb[:, ts(h, DK)], v_ps[:, ts(h, DK)]
                        )

                    # ---- Phase 2b: att_h = probs_h @ v_h (pre-normalized) --
                    att_sb = []
                    for h in range(HPC):
                        att_ps = psB.tile([P, P], F32, tag="t")
                        nc.tensor.matmul(
                            att_ps,
                            pT[h],
                            v_sb[:, ts(h, DK)],
                            start=True,
                            stop=True,
                        )
                        a_sb = sb.tile([P, DK], BF16, tag=f"att{h}")
                        nc.vector.tensor_copy(a_sb, att_ps)
                        att_sb.append(a_sb)

                    # ---- Phase 3: out^T[d, dm] = sum_h att_h^T @ OT chunk.
                    # Chunks pair up into bf16 [128, 1024] stores (2KB DMA
                    # lines); all pair tiles stay resident so the store queue
                    # never backpressures the PE via PSUM-copy recycling.
                    for c in range(NCHUNK):
                        o_ps = psC.tile([P, OW], F32, tag="o")
                        for h in range(HPC):
                            nc.tensor.matmul(
                                o_ps,
                                att_sb[h],
                                ot_c[c][:, ts(h, OW)],
                                start=(h == 0),
                                stop=(h == HPC - 1),
                            )
                        if c % 2 == 0:
                            o_pair = outp.tile(
                                [P, 2 * OW], BF16, tag=f"o_pair{c // 2}"
                            )
                        nc.vector.tensor_copy(o_pair[:, ts(c % 2, OW)], o_ps)
                        if c % 2 == 1:
                            # sync queue: in-order behind all loads, so the
                            # load stream stays pure-read (no HBM write
                            # turnarounds mid-stream)
                            nc.sync.dma_start(out[:, ts(c // 2, 2 * OW)], o_pair)

    nc.compile()
    return nc


def _pt(a):
    """(n*128, W) -> partition-tiled (128, n*W): res[p, i*W+c] = a[i*128+p, c]."""
    n = a.shape[0] // P
    return np.ascontiguousarray(
        a.reshape(n, P, a.shape[1]).transpose(1, 0, 2).reshape(P, -1)
    )


def make_in_maps(Q, K, V, O, x):
    Q = np.asarray(Q, dtype=np.float32)
    K = np.asarray(K, dtype=np.float32)
    V = np.asarray(V, dtype=np.float32)
    O = np.asarray(O, dtype=np.float32)
    x = np.asarray(x, dtype=np.float32)
    xtb = _pt(np.ascontiguousarray(x.T).astype(BF))
    in_maps = []
    for c in range(NCORES):
        sl = slice(c * OW, (c + 1) * OW)
        ot = np.ascontiguousarray(O[:, sl].T).astype(BF)  # (OW, DM)
        otb = np.ascontiguousarray(
            ot.reshape(HPC, P, NCHUNK, OW).transpose(1, 2, 0, 3).reshape(P, -1)
        )
        in_maps.append(
            {
                "qtb": _pt(np.ascontiguousarray(Q[sl].T).astype(BF)),
                "ktb": _pt(np.ascontiguousarray(K[sl].T).astype(BF)),
                "vtb": _pt(np.ascontiguousarray(V[sl].T).astype(BF)),
                "otb": otb,
                "xtb": xtb,
            }
        )
    return in_maps


_NC_CACHE = {}


def _get_nc():
    if "nc" not in _NC_CACHE:
        _NC_CACHE["nc"] = build_nc()
    return _NC_CACHE["nc"]


def kernel(Q, K, V, O, x, _trace=False):
    nc = _get_nc()
    in_maps = make_in_maps(Q, K, V, O, x)
    res = run_bass_kernel_spmd(
        nc, in_maps, core_ids=list(range(NCORES)), trace=_trace
    )
    acc = np.zeros((SEQ, DM), dtype=np.float64)
    for c in range(NCORES):
        acc += res.results[c]["out"].astype(np.float64)
    outT = acc.astype(np.float32)
    if _trace:
        kernel.last_exec_time_ns = res.exec_time_ns
        kernel.last_results = res
    return np.ascontiguousarray(outT.T)

